# revision 27
# baseline (speedup 1.0000x reference)
"""Multi-head attention (B=2, S=2048, D=1024, 16 heads x 64) on 8 TRN2 cores.

Sharding: batch x head-group. Core c owns batch b = c//4 and head group
g = c%4 (4 heads, W-rows [256g, 256g+256)). Core output is the (2048, 256)
feature slice; host assembles [B, S, D]. No collectives.

Per-core pipeline (bf16 matmul operands, f32 PSUM):
  x, W: transposed + cast to bf16 on the HOST into the exact SBUF
  layouts (pure data marshalling; all FLOPs stay on device), DMA'd
  straight into xt/wt - no PE transposes, no on-device casts.
  q,k proj as [w, s] (lhsT = W.T chunk, rhs = xt). v proj as [s, w]
  (lhsT = xt chunk, rhs = Wv.T) -> v2[t, h, 65] with em[t]-scaled values
  and em[t] in column 64 (em = exp(1e4*mask - 1e4) folds the additive
  mask exactly; the 65th column makes PV also produce the softmax
  denominator Z). Projection biases are zeros by problem spec; skipped.
  Attention: 8 blocks (2 head pairs x 4 s-blocks of 512), software-
  pipelined one block deep: block k runs QK+exp while PV of block k-1
  consumes its stashed et tiles (lagged one extra tc so finalize's DVE
  copies never head-of-line-block the PE), keeping ACT (the bottleneck:
  1024-row exp ~1.0us, 128 calls ~130us) dense.
  QK: two row-tiled K=64 matmuls, tile_position (0,0)/(64,0), run
  concurrently on the PE (measured 1.7x). PV in "swap" form: out[s=128,
  65] = et-chunk.T @ v2[t, 65] at full PE utilization (measured 32ns).
  PV start=True only on each bank's first matmul: start clears the
  WHOLE bank's has_written bits, so later first-touches initialize.
  Later q/k projection segments are interleaved into the attention loop
  ("borrows" of a psc PSUM slot) inside the PE slack.
  Finalize: DVE copies ph -> SBUF (all four banks first), reciprocal +
  scale on DVE, HWDGE DMA out.
"""

import sys

if "/opt/trn_rl_repo" not in sys.path:
    sys.path.insert(0, "/opt/trn_rl_repo")

import numpy as np

B = 2
S = 2048
D = 1024
NCORES = 8
WC = 256          # per-core projection width (4 heads x 64)
NH = 4            # heads per core
NP = 2            # head pairs per core
W = 64            # head dim
KC = D // 128     # contraction chunks (8)
SC = S // 128     # 128-row chunks of S (16)
SEG = 512         # proj segment
NSEG = S // SEG   # 4
SBLK = 512        # attention s-block
NBLK = S // SBLK  # 4


def _build():
    import concourse.tile as tile
    from concourse import bacc, mybir

    f32 = mybir.dt.float32
    bf16 = mybir.dt.bfloat16
    EXP = mybir.ActivationFunctionType.Exp
    MUL = mybir.AluOpType.mult

    nc = bacc.Bacc("TRN2", target_bir_lowering=False, debug=False)

    xt_d = nc.dram_tensor("xt", [128, KC * S], bf16, kind="ExternalInput")
    m_d = nc.dram_tensor("m", [S], f32, kind="ExternalInput")
    wq_d = nc.dram_tensor("wq", [128, KC * WC], bf16, kind="ExternalInput")
    wk_d = nc.dram_tensor("wk", [128, KC * WC], bf16, kind="ExternalInput")
    wv_d = nc.dram_tensor("wv", [128, KC * WC], bf16, kind="ExternalInput")
    o_d = nc.dram_tensor("out", [S, WC], f32, kind="ExternalOutput")

    with tile.TileContext(nc) as tc:
        consts = tc.alloc_tile_pool(name="consts", bufs=1)
        etp = tc.alloc_tile_pool(name="etp", bufs=2 * SC)
        hsp = tc.alloc_tile_pool(name="hsp", bufs=4)
        otp = tc.alloc_tile_pool(name="otp", bufs=4)
        ps_qk = tc.alloc_tile_pool(name="ps_qk", bufs=2, space="PSUM")
        ps_ph = tc.alloc_tile_pool(name="ps_ph", bufs=4, space="PSUM")

        # persistent SBUF tensors
        xt = consts.tile([128, NSEG, KC, SEG], bf16, tag="xt")   # x.T, seg-major
        wts = {n: consts.tile([128, KC, WC], bf16, tag=f"wt_{n}", name=f"wt_{n}")
               for n in ("q", "k", "v")}
        qt = consts.tile([128, NP, S], bf16, tag="qt")
        kt = consts.tile([128, NP, S], bf16, tag="kt")
        v2 = consts.tile([128, SC, NH, W + 1], bf16, tag="v2")
        em = consts.tile([128, SC], f32, tag="em")

        # --- input DMAs: k/q weights first (critical path), then xt in
        # 4 segment-groups, then v weights; all pre-transposed bf16 ---
        nc.sync.dma_start(
            out=wts["k"][:, :, :],
            in_=wk_d[:, :].rearrange("p (kc w) -> p kc w", w=WC))
        nc.scalar.dma_start(
            out=wts["q"][:, :, :],
            in_=wq_d[:, :].rearrange("p (kc w) -> p kc w", w=WC))
        xt_dv = xt_d[:, :].rearrange("p (sg kc s) -> p sg kc s", kc=KC, s=SEG)
        hwdge = [nc.sync, nc.scalar]
        for kc in range(KC):
            hwdge[kc % 2].dma_start(out=xt[:, 0, kc, :], in_=xt_dv[:, 0, kc, :])
        nc.sync.dma_start(
            out=wts["v"][:, :, :],
            in_=wv_d[:, :].rearrange("p (kc w) -> p kc w", w=WC))
        nc.scalar.dma_start(out=xt[:, 1, :, :], in_=xt_dv[:, 1, :, :])
        nc.sync.dma_start(out=xt[:, 2, :, :], in_=xt_dv[:, 2, :, :])
        nc.scalar.dma_start(out=xt[:, 3, :, :], in_=xt_dv[:, 3, :, :])

        msk = consts.tile([128, SC], f32, tag="msk")
        nc.gpsimd.dma_start(out=msk[:, :], in_=m_d.ap().rearrange("(c p) -> p c", p=128))
        mb = consts.tile([128, 1], f32, tag="mb")
        nc.vector.memset(mb[:, :], -10000.0)
        # em[t] = exp(1e4*mask - 1e4)  (1 for kept keys, ~0 for masked)
        nc.scalar.activation(em[:, :], msk[:, :], EXP, scale=10000.0, bias=mb[:, :])

        # v2 Z columns = em (bf16 cast)
        for h in range(NH):
            nc.vector.tensor_copy(
                v2[:, :, h, W:W + 1],
                em[:, :].rearrange("p (c one) -> p c one", one=1))

        def proj_seg(dst, wname, pair, sseg):
            """dst[:, pair, sseg*512:...] = (W.T chunks @ xt) for one segment."""
            pp = ps_qk.tile([128, 512], f32, tag="psc", name="pp")
            wt = wts[wname]
            for kc in range(KC):
                nc.tensor.matmul(
                    pp[:, :],
                    lhsT=wt[:, kc, pair * 128:(pair + 1) * 128],
                    rhs=xt[:, sseg, kc, :],
                    start=(kc == 0), stop=(kc == KC - 1),
                )
            nc.vector.tensor_copy(dst[:, pair, sseg * SEG:(sseg + 1) * SEG], pp[:, :])

        def vproj_sc(sc):
            """v2[:, sc, h, 0:64] = em[sc] * (x @ Wv.T)[sc-chunk] (as [s, w'])."""
            pv = ps_ph.tile([128, 512], f32, tag="ph", name="pv")
            for kc in range(KC):
                nc.tensor.matmul(
                    pv[:, 0:WC],
                    lhsT=xt[:, sc // 4, kc, (sc % 4) * 128:(sc % 4 + 1) * 128],
                    rhs=wts["v"][:, kc, :],
                    start=(kc == 0), stop=(kc == KC - 1),
                )
            nc.vector.tensor_scalar(
                out=v2[:, sc, :, 0:W],
                in0=pv[:, 0:WC].rearrange("p (h w) -> p h w", h=NH),
                scalar1=em[:, sc:sc + 1], scalar2=None, op0=MUL,
            )

        # first k/q segments as soon as xt segment 0 lands; the remaining
        # k-proj segments stream inside block 0's loop
        proj_seg(kt, "k", 0, 0)
        proj_seg(qt, "q", 0, 0)

        # --- attention: 8 blocks, PV pipelined one block + one tc behind ---
        def qk_mms(psc, pair, blk, tcc):
            for j in range(2):
                nc.tensor.matmul(
                    psc[:, j, :],
                    lhsT=kt[j * W:(j + 1) * W, pair, tcc * 128:(tcc + 1) * 128],
                    rhs=qt[j * W:(j + 1) * W, pair, blk * SBLK:(blk + 1) * SBLK],
                    start=True, stop=True,
                )

        def pv_mms(ph, pair, tcc, et):
            # start=True only on each bank's first matmul: it clears the
            # whole bank's has_written bits, so every element's first write
            # initializes (including the other head's region)
            for j in range(2):
                h = pair * 2 + j
                for sc4 in range(4):
                    nc.tensor.matmul(
                        ph[sc4][:, j, 0:W + 1],
                        lhsT=et[:, j, sc4 * 128:(sc4 + 1) * 128],
                        rhs=v2[:, tcc, h, :],
                        start=(tcc == 0 and j == 0), stop=(tcc == SC - 1),
                        skip_group_check=True,
                    )

        def alloc_ph():
            # per-head stride 66 f32: disjoint 8-byte PSUM cachelines
            ph_f = [ps_ph.tile([128, 512], f32, tag="ph", name="ph")
                    for _ in range(4)]
            return [p[:, 0:2 * (W + 2)].rearrange("p (h w) -> p h w", w=W + 2)
                    for p in ph_f]

        def finalize(ph, pair, blk):
            # h = ph[:, j, 0:64] / Z, Z = ph[:, j, 64]; all PSUM->SBUF copies
            # first so the next block's ph reuse isn't gated on the muls
            hsbs = []
            for sc4 in range(4):
                hsb = hsp.tile([128, 2, W + 2], f32, tag="hsb")
                nc.vector.tensor_copy(hsb[:, :, :], ph[sc4][:, :, :])
                hsbs.append(hsb)
            for sc4 in range(4):
                hsb = hsbs[sc4]
                rec = otp.tile([128, 2], f32, tag="rec")
                nc.vector.reciprocal(
                    rec[:, :], hsb[:, :, W:W + 1].rearrange("p h one -> p (h one)"))
                ot = otp.tile([128, 2 * W], f32, tag="ot")
                for j in range(2):
                    nc.vector.tensor_scalar(
                        out=ot[:, j * W:(j + 1) * W],
                        in0=hsb[:, j, 0:W],
                        scalar1=rec[:, j:j + 1],
                        scalar2=None, op0=MUL,
                    )
                s0 = blk * SBLK + sc4 * 128
                (nc.sync if sc4 % 2 == 0 else nc.scalar).dma_start(
                    out=o_d[s0:s0 + 128, pair * 128:(pair + 1) * 128],
                    in_=ot[:, :])

        def borrow_pieces(dst, wname, pair, sseg):
            """proj_seg split into two half-contraction pieces emitted at
            consecutive tccs: each PE lump fits the per-tcc slack, so the
            borrow no longer starves ACT. Holding the psc slot across one
            intervening QK allocation is safe with the 2-slot ring."""
            st = {}
            def piece(i):
                if i == 0:
                    st["pp"] = ps_qk.tile([128, 512], f32, tag="psc", name="pp")
                pp = st["pp"]
                wt = wts[wname]
                for kc in range(4 * i, 4 * i + 4):
                    nc.tensor.matmul(
                        pp[:, :],
                        lhsT=wt[:, kc, pair * 128:(pair + 1) * 128],
                        rhs=xt[:, sseg, kc, :],
                        start=(kc == 0), stop=(kc == KC - 1),
                    )
                if i == 1:
                    nc.vector.tensor_copy(
                        dst[:, pair, sseg * SEG:(sseg + 1) * SEG], pp[:, :])
            return piece

        blocks = [(pair, blk) for pair in range(NP) for blk in range(NBLK)]
        # two borrows per block, each split in two pieces; block 4's
        # kproj(1,3) finishes at tcc 10, before its tcc-12 QK needs it
        borrows = {
            1: [("q", 0, 2), ("k", 1, 0)],
            2: [("q", 0, 3), ("k", 1, 1)],
            3: [("q", 1, 0), ("k", 1, 2)],
            4: [("q", 1, 1), ("k", 1, 3)],
            5: [("q", 1, 2)],
            6: [("q", 1, 3)],
        }
        prev = None
        for bi, (pair, blk) in enumerate(blocks):
            ph = alloc_ph() if prev is not None else None
            ets = []
            bb = borrows.get(bi, [])
            sched = {}
            if bb:
                p0 = borrow_pieces(kt if bb[0][0] == "k" else qt, *bb[0])
                sched[4], sched[5] = (p0, 0), (p0, 1)
                if len(bb) > 1:
                    p1 = borrow_pieces(kt if bb[1][0] == "k" else qt, *bb[1])
                    t0 = 9 if bi == 4 else 10
                    sched[t0], sched[t0 + 1] = (p1, 0), (p1, 1)
            for tcc in range(SC):
                psc = ps_qk.tile([128, 2, 512], f32, tag="psc", name="psc")
                qk_mms(psc, pair, blk, tcc)
                et = etp.tile([128, 2, 512], bf16, tag="et")
                nc.scalar.activation(et[:, :, :], psc[:, :, :], EXP, scale=0.125)
                ets.append(et)
                if prev is not None and tcc >= 1:
                    pv_mms(ph, prev[0], tcc - 1, prev[2][tcc - 1])
                if tcc in sched:
                    fn, i = sched[tcc]
                    fn(i)
                if bi == 0:
                    # stream the rest of prep inside block 0
                    vproj_sc(tcc)
                    if tcc in (1, 5, 9):
                        proj_seg(kt, "k", 0, tcc // 4 + 1)
                    if tcc == 12:
                        proj_seg(qt, "q", 0, 1)
            if prev is not None:
                pv_mms(ph, prev[0], SC - 1, prev[2][SC - 1])
                finalize(ph, prev[0], prev[1])
            prev = (pair, blk, ets)
        # drain: PV + finalize of the last block
        ph = alloc_ph()
        for tcc in range(SC):
            pv_mms(ph, prev[0], tcc, prev[2][tcc])
        finalize(ph, prev[0], prev[1])

        for p in (ps_ph, ps_qk, otp, hsp, etp, consts):
            p.release()

    nc.finalize()
    return nc


_NC = None


def _get_nc():
    global _NC
    if _NC is None:
        _NC = _build()
    return _NC


def _tr(a):
    # [R, D] f32 -> bf16 [128, KC * R] in the SBUF layout t[p, kc, r] =
    # a[r, kc*128 + p]
    import ml_dtypes
    R = a.shape[0]
    t = a.T.reshape(KC, 128, R).transpose(1, 0, 2).reshape(128, KC * R)
    return np.ascontiguousarray(t.astype(ml_dtypes.bfloat16))


def _tr_x(a):
    # [S, D] f32 -> bf16 [128, S*KC] seg-major: t[p, sg, kc, s'] =
    # a[sg*512 + s', kc*128 + p] (one contiguous descriptor per partition
    # and segment)
    import ml_dtypes
    t = a.T.reshape(KC, 128, NSEG, SEG).transpose(1, 2, 0, 3).reshape(128, KC * S)
    return np.ascontiguousarray(t.astype(ml_dtypes.bfloat16))


def _in_maps(inputs):
    x = np.asarray(inputs["hidden_states"], dtype=np.float32)
    m = np.asarray(inputs["attn_mask"], dtype=np.float32)
    wq = np.asarray(inputs["Wq"], dtype=np.float32)
    wk = np.asarray(inputs["Wk"], dtype=np.float32)
    wv = np.asarray(inputs["Wv"], dtype=np.float32)
    maps = []
    for c in range(NCORES):
        b, g = c // 4, c % 4
        sl = slice(g * WC, (g + 1) * WC)
        maps.append({
            "xt": _tr_x(x[b]),
            "m": np.ascontiguousarray(m[b]),
            "wq": _tr(wq[sl]),
            "wk": _tr(wk[sl]),
            "wv": _tr(wv[sl]),
        })
    return maps


def _run(inputs, trace=False):
    from concourse.bass_utils import run_bass_kernel_spmd

    nc = _get_nc()
    res = run_bass_kernel_spmd(
        nc, _in_maps(inputs), core_ids=list(range(NCORES)), trace=trace
    )
    out = np.empty((B, S, D), dtype=np.float32)
    for c in range(NCORES):
        b, g = c // 4, c % 4
        out[b, :, g * WC:(g + 1) * WC] = res.results[c]["out"]
    return out, res


def kernel(**inputs):
    out, _ = _run(inputs, trace=False)
    return out


# revision 28
# speedup vs baseline: 1.1758x; 1.1758x over previous
"""Multi-head attention (B=2, S=2048, D=1024, 16 heads x 64) on 8 TRN2 cores.

Sharding: batch x head-group. Core c owns batch b = c//4 and head group
g = c%4 (4 heads, W-rows [256g, 256g+256)). Core output is the (2048, 256)
feature slice; host assembles [B, S, D]. No collectives.

Per-core pipeline (bf16 matmul operands, f32 PSUM):
  x, W: transposed + cast to bf16 on the HOST into the exact SBUF
  layouts (pure data marshalling; all FLOPs stay on device), DMA'd
  straight into xt/wt - no PE transposes, no on-device casts.
  q,k proj as [w, s] (lhsT = W.T chunk, rhs = xt). v proj as [s, w]
  (lhsT = xt chunk, rhs = Wv.T) -> v2[t, h, 65] with em[t]-scaled values
  and em[t] in column 64 (em = exp(1e4*mask - 1e4) folds the additive
  mask exactly; the 65th column makes PV also produce the softmax
  denominator Z). Projection biases are zeros by problem spec; skipped.
  Attention: 8 blocks (2 head pairs x 4 s-blocks of 512), software-
  pipelined one block deep: block k runs QK+exp while PV of block k-1
  consumes its stashed et tiles (lagged one extra tc so finalize's DVE
  copies never head-of-line-block the PE), keeping ACT (the bottleneck:
  1024-row exp ~1.0us, 128 calls ~130us) dense.
  QK: two row-tiled K=64 matmuls, tile_position (0,0)/(64,0), run
  concurrently on the PE (measured 1.7x). PV in "swap" form: out[s=128,
  65] = et-chunk.T @ v2[t, 65] at full PE utilization (measured 32ns).
  PV start=True only on each bank's first matmul: start clears the
  WHOLE bank's has_written bits, so later first-touches initialize.
  Later q/k projection segments are interleaved into the attention loop
  ("borrows" of a psc PSUM slot) inside the PE slack.
  Finalize: DVE copies ph -> SBUF (all four banks first), reciprocal +
  scale on DVE, HWDGE DMA out.
"""

import sys

if "/opt/trn_rl_repo" not in sys.path:
    sys.path.insert(0, "/opt/trn_rl_repo")

import numpy as np

B = 2
S = 2048
D = 1024
NCORES = 8
WC = 256          # per-core projection width (4 heads x 64)
NH = 4            # heads per core
NP = 2            # head pairs per core
W = 64            # head dim
KC = D // 128     # contraction chunks (8)
SC = S // 128     # 128-row chunks of S (16)
SEG = 512         # proj segment
NSEG = S // SEG   # 4
SBLK = 512        # attention s-block
NBLK = S // SBLK  # 4


def _build():
    import concourse.tile as tile
    from concourse import bacc, mybir

    f32 = mybir.dt.float32
    bf16 = mybir.dt.bfloat16
    EXP = mybir.ActivationFunctionType.Exp
    MUL = mybir.AluOpType.mult

    nc = bacc.Bacc("TRN2", target_bir_lowering=False, debug=False)

    xt_d = nc.dram_tensor("xt", [128, KC * S], bf16, kind="ExternalInput")
    m_d = nc.dram_tensor("m", [S], f32, kind="ExternalInput")
    wq_d = nc.dram_tensor("wq", [128, KC * WC], bf16, kind="ExternalInput")
    wk_d = nc.dram_tensor("wk", [128, KC * WC], bf16, kind="ExternalInput")
    wv_d = nc.dram_tensor("wv", [128, KC * WC], bf16, kind="ExternalInput")
    o_d = nc.dram_tensor("out", [S, WC], f32, kind="ExternalOutput")

    with tile.TileContext(nc) as tc:
        consts = tc.alloc_tile_pool(name="consts", bufs=1)
        etp = tc.alloc_tile_pool(name="etp", bufs=2 * SC)
        hsp = tc.alloc_tile_pool(name="hsp", bufs=4)
        otp = tc.alloc_tile_pool(name="otp", bufs=4)
        ps_qk = tc.alloc_tile_pool(name="ps_qk", bufs=2, space="PSUM")
        ps_ph = tc.alloc_tile_pool(name="ps_ph", bufs=4, space="PSUM")

        # persistent SBUF tensors
        xt = consts.tile([128, NSEG, KC, SEG], bf16, tag="xt")   # x.T, seg-major
        wts = {n: consts.tile([128, KC, WC], bf16, tag=f"wt_{n}", name=f"wt_{n}")
               for n in ("q", "k", "v")}
        qt = consts.tile([128, NP, S], bf16, tag="qt")
        kt = consts.tile([128, NP, S], bf16, tag="kt")
        v2 = consts.tile([128, SC, NH, W + 1], bf16, tag="v2")
        em = consts.tile([128, SC], f32, tag="em")

        # --- input DMAs: k/q weights first (critical path), then xt in
        # 4 segment-groups, then v weights; all pre-transposed bf16 ---
        nc.sync.dma_start(
            out=wts["k"][:, :, :],
            in_=wk_d[:, :].rearrange("p (kc w) -> p kc w", w=WC))
        nc.scalar.dma_start(
            out=wts["q"][:, :, :],
            in_=wq_d[:, :].rearrange("p (kc w) -> p kc w", w=WC))
        xt_dv = xt_d[:, :].rearrange("p (sg kc s) -> p sg kc s", kc=KC, s=SEG)
        for kc in range(KC):
            nc.sync.dma_start(out=xt[:, 0, kc, :], in_=xt_dv[:, 0, kc, :])
        nc.scalar.dma_start(out=xt[:, 1, :, :], in_=xt_dv[:, 1, :, :])
        nc.sync.dma_start(
            out=wts["v"][:, :, :],
            in_=wv_d[:, :].rearrange("p (kc w) -> p kc w", w=WC))
        nc.scalar.dma_start(out=xt[:, 3, :, :], in_=xt_dv[:, 3, :, :])
        nc.sync.dma_start(out=xt[:, 2, :, :], in_=xt_dv[:, 2, :, :])

        msk = consts.tile([128, SC], f32, tag="msk")
        nc.gpsimd.dma_start(out=msk[:, :], in_=m_d.ap().rearrange("(c p) -> p c", p=128))
        mb = consts.tile([128, 1], f32, tag="mb")
        nc.vector.memset(mb[:, :], -10000.0)
        # em[t] = exp(1e4*mask - 1e4)  (1 for kept keys, ~0 for masked)
        nc.scalar.activation(em[:, :], msk[:, :], EXP, scale=10000.0, bias=mb[:, :])

        # v2 Z columns = em (bf16 cast)
        for h in range(NH):
            nc.vector.tensor_copy(
                v2[:, :, h, W:W + 1],
                em[:, :].rearrange("p (c one) -> p c one", one=1))

        def proj_seg(dst, wname, pair, sseg):
            """dst[:, pair, sseg*512:...] = (W.T chunks @ xt) for one segment."""
            pp = ps_qk.tile([128, 512], f32, tag="psc", name="pp")
            wt = wts[wname]
            for kc in range(KC):
                nc.tensor.matmul(
                    pp[:, :],
                    lhsT=wt[:, kc, pair * 128:(pair + 1) * 128],
                    rhs=xt[:, sseg, kc, :],
                    start=(kc == 0), stop=(kc == KC - 1),
                )
            nc.vector.tensor_copy(dst[:, pair, sseg * SEG:(sseg + 1) * SEG], pp[:, :])

        def vproj_sc(sc):
            """v2[:, sc, h, 0:64] = em[sc] * (x @ Wv.T)[sc-chunk] (as [s, w'])."""
            pv = ps_ph.tile([128, 512], f32, tag="ph", name="pv")
            for kc in range(KC):
                nc.tensor.matmul(
                    pv[:, 0:WC],
                    lhsT=xt[:, sc // 4, kc, (sc % 4) * 128:(sc % 4 + 1) * 128],
                    rhs=wts["v"][:, kc, :],
                    start=(kc == 0), stop=(kc == KC - 1),
                )
            nc.vector.tensor_scalar(
                out=v2[:, sc, :, 0:W],
                in0=pv[:, 0:WC].rearrange("p (h w) -> p h w", h=NH),
                scalar1=em[:, sc:sc + 1], scalar2=None, op0=MUL,
            )

        # first k/q segments as soon as xt segment 0 lands; the remaining
        # k-proj segments stream inside block 0's loop
        proj_seg(kt, "k", 0, 0)
        proj_seg(qt, "q", 0, 0)

        # --- attention: 8 blocks, PV pipelined one block + one tc behind ---
        def qk_mms(psc, pair, blk, tcc):
            for j in range(2):
                nc.tensor.matmul(
                    psc[:, j, :],
                    lhsT=kt[j * W:(j + 1) * W, pair, tcc * 128:(tcc + 1) * 128],
                    rhs=qt[j * W:(j + 1) * W, pair, blk * SBLK:(blk + 1) * SBLK],
                    start=True, stop=True,
                )

        def pv_mms(ph, pair, tcc, et):
            # start=True only on each bank's first matmul: it clears the
            # whole bank's has_written bits, so every element's first write
            # initializes (including the other head's region)
            for j in range(2):
                h = pair * 2 + j
                for sc4 in range(4):
                    nc.tensor.matmul(
                        ph[sc4][:, j, 0:W + 1],
                        lhsT=et[:, j, sc4 * 128:(sc4 + 1) * 128],
                        rhs=v2[:, tcc, h, :],
                        start=(tcc == 0 and j == 0), stop=(tcc == SC - 1),
                        skip_group_check=True,
                    )

        def alloc_ph():
            # per-head stride 66 f32: disjoint 8-byte PSUM cachelines
            ph_f = [ps_ph.tile([128, 512], f32, tag="ph", name="ph")
                    for _ in range(4)]
            return [p[:, 0:2 * (W + 2)].rearrange("p (h w) -> p h w", w=W + 2)
                    for p in ph_f]

        def finalize(ph, pair, blk):
            # h = ph[:, j, 0:64] / Z, Z = ph[:, j, 64]; all PSUM->SBUF copies
            # first so the next block's ph reuse isn't gated on the muls
            hsbs = []
            for sc4 in range(4):
                hsb = hsp.tile([128, 2, W + 2], f32, tag="hsb")
                nc.vector.tensor_copy(hsb[:, :, :], ph[sc4][:, :, :])
                hsbs.append(hsb)
            for sc4 in range(4):
                hsb = hsbs[sc4]
                rec = otp.tile([128, 2], f32, tag="rec")
                nc.vector.reciprocal(
                    rec[:, :], hsb[:, :, W:W + 1].rearrange("p h one -> p (h one)"))
                ot = otp.tile([128, 2 * W], f32, tag="ot")
                for j in range(2):
                    nc.vector.tensor_scalar(
                        out=ot[:, j * W:(j + 1) * W],
                        in0=hsb[:, j, 0:W],
                        scalar1=rec[:, j:j + 1],
                        scalar2=None, op0=MUL,
                    )
                s0 = blk * SBLK + sc4 * 128
                nc.sync.dma_start(
                    out=o_d[s0:s0 + 128, pair * 128:(pair + 1) * 128],
                    in_=ot[:, :])

        def borrow_pieces(dst, wname, pair, sseg):
            """proj_seg split into two half-contraction pieces emitted at
            consecutive tccs: each PE lump fits the per-tcc slack, so the
            borrow no longer starves ACT. Holding the psc slot across one
            intervening QK allocation is safe with the 2-slot ring."""
            st = {}
            def piece(i):
                if i == 0:
                    st["pp"] = ps_qk.tile([128, 512], f32, tag="psc", name="pp")
                pp = st["pp"]
                wt = wts[wname]
                for kc in range(4 * i, 4 * i + 4):
                    nc.tensor.matmul(
                        pp[:, :],
                        lhsT=wt[:, kc, pair * 128:(pair + 1) * 128],
                        rhs=xt[:, sseg, kc, :],
                        start=(kc == 0), stop=(kc == KC - 1),
                    )
                if i == 1:
                    nc.vector.tensor_copy(
                        dst[:, pair, sseg * SEG:(sseg + 1) * SEG], pp[:, :])
            return piece

        blocks = [(pair, blk) for pair in range(NP) for blk in range(NBLK)]
        # two borrows per block, each split in two pieces; block 4's
        # kproj(1,3) finishes at tcc 10, before its tcc-12 QK needs it
        borrows = {
            1: [("q", 0, 2), ("k", 1, 0)],
            2: [("q", 0, 3), ("k", 1, 1)],
            3: [("q", 1, 0), ("k", 1, 2)],
            4: [("q", 1, 1), ("k", 1, 3)],
            5: [("q", 1, 2)],
            6: [("q", 1, 3)],
        }
        prev = None
        for bi, (pair, blk) in enumerate(blocks):
            ph = alloc_ph() if prev is not None else None
            ets = []
            bb = borrows.get(bi, [])
            sched = {}
            if bb:
                p0 = borrow_pieces(kt if bb[0][0] == "k" else qt, *bb[0])
                sched[4], sched[5] = (p0, 0), (p0, 1)
                if len(bb) > 1:
                    p1 = borrow_pieces(kt if bb[1][0] == "k" else qt, *bb[1])
                    t0 = 9 if bi == 4 else 10
                    sched[t0], sched[t0 + 1] = (p1, 0), (p1, 1)
            for tcc in range(SC):
                psc = ps_qk.tile([128, 2, 512], f32, tag="psc", name="psc")
                qk_mms(psc, pair, blk, tcc)
                et = etp.tile([128, 2, 512], bf16, tag="et")
                nc.scalar.activation(et[:, :, :], psc[:, :, :], EXP, scale=0.125)
                ets.append(et)
                if prev is not None and tcc >= 1:
                    pv_mms(ph, prev[0], tcc - 1, prev[2][tcc - 1])
                if tcc in sched:
                    fn, i = sched[tcc]
                    fn(i)
                if bi == 0:
                    # stream the rest of prep inside block 0
                    vproj_sc(tcc)
                    if tcc in (1, 5, 9):
                        proj_seg(kt, "k", 0, tcc // 4 + 1)
                    if tcc == 12:
                        proj_seg(qt, "q", 0, 1)
            if prev is not None:
                pv_mms(ph, prev[0], SC - 1, prev[2][SC - 1])
                finalize(ph, prev[0], prev[1])
            prev = (pair, blk, ets)
        # drain: PV + finalize of the last block
        ph = alloc_ph()
        for tcc in range(SC):
            pv_mms(ph, prev[0], tcc, prev[2][tcc])
        finalize(ph, prev[0], prev[1])

        for p in (ps_ph, ps_qk, otp, hsp, etp, consts):
            p.release()

    nc.finalize()
    return nc


_NC = None


def _get_nc():
    global _NC
    if _NC is None:
        _NC = _build()
    return _NC


def _tr(a):
    # [R, D] f32 -> bf16 [128, KC * R] in the SBUF layout t[p, kc, r] =
    # a[r, kc*128 + p]
    import ml_dtypes
    R = a.shape[0]
    t = a.T.reshape(KC, 128, R).transpose(1, 0, 2).reshape(128, KC * R)
    return np.ascontiguousarray(t.astype(ml_dtypes.bfloat16))


def _tr_x(a):
    # [S, D] f32 -> bf16 [128, S*KC] seg-major: t[p, sg, kc, s'] =
    # a[sg*512 + s', kc*128 + p] (one contiguous descriptor per partition
    # and segment)
    import ml_dtypes
    t = a.T.reshape(KC, 128, NSEG, SEG).transpose(1, 2, 0, 3).reshape(128, KC * S)
    return np.ascontiguousarray(t.astype(ml_dtypes.bfloat16))


def _in_maps(inputs):
    x = np.asarray(inputs["hidden_states"], dtype=np.float32)
    m = np.asarray(inputs["attn_mask"], dtype=np.float32)
    wq = np.asarray(inputs["Wq"], dtype=np.float32)
    wk = np.asarray(inputs["Wk"], dtype=np.float32)
    wv = np.asarray(inputs["Wv"], dtype=np.float32)
    maps = []
    for c in range(NCORES):
        b, g = c // 4, c % 4
        sl = slice(g * WC, (g + 1) * WC)
        maps.append({
            "xt": _tr_x(x[b]),
            "m": np.ascontiguousarray(m[b]),
            "wq": _tr(wq[sl]),
            "wk": _tr(wk[sl]),
            "wv": _tr(wv[sl]),
        })
    return maps


def _run(inputs, trace=False):
    from concourse.bass_utils import run_bass_kernel_spmd

    nc = _get_nc()
    res = run_bass_kernel_spmd(
        nc, _in_maps(inputs), core_ids=list(range(NCORES)), trace=trace
    )
    out = np.empty((B, S, D), dtype=np.float32)
    for c in range(NCORES):
        b, g = c // 4, c % 4
        out[b, :, g * WC:(g + 1) * WC] = res.results[c]["out"]
    return out, res


def kernel(**inputs):
    out, _ = _run(inputs, trace=False)
    return out
